# revision 1
# baseline (speedup 1.0000x reference)
"""Lorentz multi-head attention on 8 Trainium2 NeuronCores.

Sharding: head-parallel phase 1 (core c computes head c for all batches:
QKV Lorentz projections, Lorentz-inner-product scores, softmax-free
exp-attention, Lorentz-midpoint normalize), then an AllToAll exchanges
(head-block -> token-block) so phase 2 (concat_logradius fusion + output
LorentzFC) runs token-parallel (core c handles 1024 of the 8192 tokens).

Softmax denominator is skipped entirely: the Lorentz midpoint renormalizes
m / sqrt(K*(t^2-||s||^2)), which is invariant to positive row scaling, so
exp(scores) can be used unnormalized (scores are O(+-5), no overflow risk).

Biases are folded into the matmuls by augmenting tokens with a constant-1
column and weights with a bias row. sqrt/rsqrt are computed as
exp(+-0.5*ln(x)) so the ScalarEngine needs only the one
natural_log_exp_and_others table set (no ~2.7us table swaps).

Big matmuls run in float32r (4x the fp32 rate at moving-dim >= 256);
attention output is accumulated transposed ([65, n]) so its moving dim is
512, then rotated back 128 tokens at a time through the PE transpose path.
"""

import os
import sys

sys.path.insert(0, "/opt/trn_rl_repo")

import numpy as np

_SKIP_CC = os.environ.get("LA_SKIP_CC", "0") == "1"  # debug: phase 1 only
_SKIP_P1 = os.environ.get("LA_SKIP_P1", "0") == "1"  # debug: phase 2 only

import concourse.bass as bass
import concourse.mybir as mybir
import concourse.tile as tile
from concourse import bacc, bass_utils
from concourse.masks import make_identity

# Problem constants (hardcoded per task contract)
B, N, D = 4, 2048, 513
H, DHS = 8, 64
NCORES = 8
KCURV = 0.1
INVK = 10.0
SCALE = 1.0 / np.sqrt(DHS)  # 0.125
S_CONST = 2.8479428291320801  # exp(0.5*(digamma(256)-digamma(32)))
DPAD = 640  # 513 padded to 5*128 (col 513 = constant-1 bias lane)
KC = 5  # contraction chunks of 128
BN = B * N  # 8192 tokens
RPC = BN // NCORES  # 1024 rows per core in phase 2
F32 = mybir.dt.float32
BF16 = mybir.dt.bfloat16
Ln = mybir.ActivationFunctionType.Ln
Exp = mybir.ActivationFunctionType.Exp

_CACHE = {}




def _patch_act_tables(nc):
    # Exp and Ln both live in the natural_log_exp_and_others set; the
    # table-load pass picks the first set containing each function, which
    # splits them across two sets and reloads tables on every Ln<->Exp
    # switch (~1.3us each). Restrict the map so the combined set wins.
    from concourse.hw_specs import get_activation_tables

    try:
        tabs = get_activation_tables(nc.m.arch)
    except Exception:
        return
    if "natural_log_exp_and_others" not in tabs:
        return
    for name, fns in tabs.items():
        if name != "natural_log_exp_and_others":
            fns.discard(Exp)
            fns.discard(Ln)


def _build():
    nc = bacc.Bacc(
        "TRN2", target_bir_lowering=False, debug=False, num_devices=NCORES
    )
    _patch_act_tables(nc)

    xT_ap = nc.dram_tensor("xT", [DPAD, BN], F32, kind="ExternalInput").ap()
    wqT_ap = nc.dram_tensor("wqT", [DPAD, DHS], F32, kind="ExternalInput").ap()
    wkT_ap = nc.dram_tensor("wkT", [DPAD, DHS], F32, kind="ExternalInput").ap()
    wvT_ap = nc.dram_tensor("wvT", [DPAD, DHS], F32, kind="ExternalInput").ap()
    woT_ap = nc.dram_tensor("woT", [DPAD, D - 1], F32, kind="ExternalInput").ap()
    y_ap = nc.dram_tensor("y", [RPC, D], F32, kind="ExternalOutput").ap()

    with tile.TileContext(nc) as tc:
        with (
            tc.tile_pool(name="const", bufs=1) as constp,
            tc.tile_pool(name="w", bufs=1) as wp,
            tc.tile_pool(name="xT", bufs=5) as xtp,
            tc.tile_pool(name="qk", bufs=2) as qkp,
            tc.tile_pool(name="sq", bufs=2) as sqp,
            tc.tile_pool(name="va", bufs=2) as vap,
            tc.tile_pool(name="pt", bufs=3) as ptp,
            tc.tile_pool(name="sm", bufs=2) as smp,
            tc.tile_pool(name="d2", bufs=2) as d2p,
            tc.tile_pool(name="ps", bufs=3, space="PSUM") as psp,
            tc.tile_pool(name="acc", bufs=1, space="PSUM") as accp,
            tc.tile_pool(name="dram", bufs=1, space="DRAM") as dramp,
        ):
            ident = constp.tile([128, 128], F32)
            make_identity(nc, ident[:])
            ones65 = constp.tile([65, 1], F32)
            nc.vector.memset(ones65[:], 1.0)
            ones65b = constp.tile([65, 1], BF16)
            nc.vector.memset(ones65b[:], 1.0)
            one1 = constp.tile([1, 1], F32)
            nc.vector.memset(one1[:], 1.0)
            bias10 = constp.tile([128, 1], F32)
            nc.vector.memset(bias10[:], INVK)
            biasD = constp.tile([128, 1], F32)
            nc.vector.memset(biasD[:], INVK * (1.0 + H * S_CONST * S_CONST))

            # Weights: [DPAD, S] viewed as [128, KC, S]
            wq = wp.tile([128, KC, DHS], F32)
            wk = wp.tile([128, KC, DHS], F32)
            wv = wp.tile([128, KC, DHS], F32)
            wo = wp.tile([128, KC, D - 1], F32)
            for w_t, w_src in ((wq, wqT_ap), (wk, wkT_ap), (wv, wvT_ap)):
                nc.sync.dma_start(
                    w_t[:], w_src.rearrange("(k p) s -> p k s", p=128)
                )
            nc.sync.dma_start(wo[:], woT_ap.rearrange("(k p) s -> p k s", p=128))
            wqb = wp.tile([128, KC, DHS], BF16)
            wkb = wp.tile([128, KC, DHS], BF16)
            wvb = wp.tile([128, KC, DHS], BF16)
            wob = wp.tile([128, KC, D - 1], BF16)
            for bf_t, f_t in ((wqb, wq), (wkb, wk), (wvb, wv), (wob, wo)):
                nc.vector.tensor_copy(bf_t[:], f_t[:])

            send = dramp.tile([BN, DHS + 1], F32)
            recv = dramp.tile([BN, DHS + 1], F32)

            # ================= Phase 1: per-batch attention =================
            for b in range(B) if not _SKIP_P1 else []:
                # ---- load xT_b chunks [128, N] x 5
                xt = []
                for ki in range(KC):
                    t = xtp.tile([128, N], F32, tag="xT", bufs=3)
                    nc.sync.dma_start(
                        t[:],
                        xT_ap[ki * 128 : (ki + 1) * 128, b * N : (b + 1) * N],
                    )
                    xt.append(t)
                xtb = []
                for ki in range(KC):
                    tb = xtp.tile([128, N], BF16, tag="xTb", name=f"xb{b}_{ki}")
                    nc.vector.tensor_copy(tb[:], xt[ki][:])
                    xtb.append(tb)

                # ---- q/k projections -> [65, N] augmented (row 64 = +-t)
                qa = qkp.tile([65, N], BF16, tag="qa")
                ka = qkp.tile([65, N], BF16, tag="ka")
                for w_t, dst, neg in ((wqb, qa, False), (wkb, ka, True)):
                    for nj in range(N // 512):
                        ps = psp.tile([64, 512], F32, tag="ps")
                        for ki in range(KC):
                            nc.tensor.matmul(
                                ps[:],
                                w_t[:, ki, :],
                                xtb[ki][:, nj * 512 : (nj + 1) * 512],
                                start=(ki == 0),
                                stop=(ki == KC - 1),
                            )
                        nc.vector.tensor_copy(
                            dst[0:64, nj * 512 : (nj + 1) * 512], ps[:]
                        )
                    # t = sqrt(INVK + sum(space^2)): ones-matmul col-sum of
                    # squares, then one Ln + one Exp over the full row
                    sq = sqp.tile([64, N], BF16, tag="sq")
                    nc.vector.tensor_mul(sq[:], dst[0:64, :], dst[0:64, :])
                    srow = smp.tile([1, N], F32, tag="row", bufs=3)
                    for nj in range(N // 512):
                        pst = psp.tile([1, 512], F32, tag="ps")
                        nc.tensor.matmul(
                            pst[:],
                            ones65b[0:64, :],
                            sq[:, nj * 512 : (nj + 1) * 512],
                            start=True,
                            stop=True,
                        )
                        nc.vector.tensor_copy(
                            srow[:, nj * 512 : (nj + 1) * 512], pst[:]
                        )
                    lrow = smp.tile([1, N], F32, tag="row", bufs=3)
                    nc.scalar.activation(lrow[:], srow[:], Ln, bias=bias10[0:1, :])
                    if neg:
                        # k gets -t so the scores matmul computes the Lorentz
                        # product q.k - t_q*t_k in one pass
                        trow = smp.tile([1, N], F32, tag="row", bufs=3)
                        nc.scalar.activation(trow[:], lrow[:], Exp, scale=0.5)
                        nc.scalar.mul(dst[64:65, :], trow[:], -1.0)
                    else:
                        nc.scalar.activation(dst[64:65, :], lrow[:], Exp, scale=0.5)

                # ---- v projection, natural layout [128, mi, 65] (col0 = t)
                va = vap.tile([128, N // 128, DHS + 1], BF16, tag="va")
                vts = smp.tile([128, N // 128, 1], F32, tag="vts")
                for mi in range(N // 128):
                    psv = psp.tile([128, 64], F32, tag="ps")
                    for ki in range(KC):
                        nc.tensor.matmul(
                            psv[:],
                            xtb[ki][:, mi * 128 : (mi + 1) * 128],
                            wvb[:, ki, :],
                            start=(ki == 0),
                            stop=(ki == KC - 1),
                        )
                    nc.vector.tensor_copy(va[:, mi, 1:65], psv[:])
                    vsq = smp.tile([128, 64], F32, tag="vsq")
                    nc.vector.tensor_mul(vsq[:], va[:, mi, 1:65], va[:, mi, 1:65])
                    nc.vector.reduce_sum(
                        vts[:, mi, :], vsq[:], axis=mybir.AxisListType.X
                    )
                # batched t_v = exp(.5 ln(sum + INVK)) for all 16 chunks
                lnv = smp.tile([128, N // 128, 1], F32, tag="lnv")
                nc.scalar.activation(lnv[:], vts[:], Ln, bias=bias10[:])
                nc.scalar.activation(va[:, :, 0:1], lnv[:], Exp, scale=0.5)

                # ---- attention: scores^T -> exp -> m^T accumulation (f32r)
                mts = []
                for nj in range(N // 512):
                    mtile = accp.tile([65, 512], F32, tag=f"acc{nj}",
                                      name=f"mts{b}_{nj}")
                    mts.append(mtile)
                for mi in range(N // 128):
                    pt = ptp.tile([128, N], BF16, tag="pt")
                    for nj in range(N // 512):
                        pss = psp.tile([128, 512], F32, tag="ps")
                        nc.tensor.matmul(
                            pss[:],
                            ka[:, mi * 128 : (mi + 1) * 128],
                            qa[:, nj * 512 : (nj + 1) * 512],
                            start=True,
                            stop=True,
                        )
                        nc.scalar.activation(
                            pt[:, nj * 512 : (nj + 1) * 512], pss[:], Exp,
                            scale=SCALE,
                        )
                    for nj in range(N // 512):
                        nc.tensor.matmul(
                            mts[nj][:],
                            va[:, mi, :],
                            pt[:, nj * 512 : (nj + 1) * 512],
                            start=(mi == 0),
                            stop=(mi == N // 128 - 1),
                        )

                # ---- Lorentz midpoint normalize (transposed layout)
                mT = sqp.tile([65, N], F32, tag="mt")
                for nj in range(N // 512):
                    nc.vector.tensor_copy(mT[:, nj * 512 : (nj + 1) * 512],
                                          mts[nj][:])
                sqT = sqp.tile([65, N], F32, tag="sq")
                nc.vector.tensor_mul(sqT[:], mT[:], mT[:])
                rT = smp.tile([1, N], F32, tag="row", bufs=3)
                for nj in range(N // 512):
                    psc = psp.tile([1, 512], F32, tag="ps")
                    nc.tensor.matmul(
                        psc[:],
                        ones65[:],
                        sqT[:, nj * 512 : (nj + 1) * 512],
                        start=True,
                        stop=True,
                    )
                    # r = 2*t^2 - sum_all(sq)  (= t^2 - ||space||^2)
                    t2c = smp.tile([1, 512], F32, tag="t2")
                    nc.vector.tensor_scalar_mul(
                        t2c[:], sqT[0:1, nj * 512 : (nj + 1) * 512], 2.0
                    )
                    nc.vector.tensor_sub(
                        rT[:, nj * 512 : (nj + 1) * 512], t2c[:], psc[:]
                    )
                # rotate r into token-partition layout via K=1 matmuls,
                # then one Ln + one Exp for all 16 chunks
                prl = psp.tile([128, N // 128], F32, tag="pr", bufs=1)
                for j in range(N // 128):
                    nc.tensor.matmul(
                        prl[:, j : j + 1],
                        rT[:, j * 128 : (j + 1) * 128],
                        one1[:],
                        start=True,
                        stop=True,
                    )
                lnr = smp.tile([128, N // 128], F32, tag="lnr")
                nc.scalar.activation(lnr[:], prl[:], Ln, scale=KCURV)
                rinv = smp.tile([128, N // 128], F32, tag="rinv")
                nc.scalar.activation(rinv[:], lnr[:], Exp, scale=-0.5)
                for nj2 in range(N // 128):
                    ptr2 = psp.tile([128, 65], F32, tag="ps")
                    nc.tensor.transpose(
                        ptr2[:], mT[:, nj2 * 128 : (nj2 + 1) * 128],
                        ident[0:65, 0:65],
                    )
                    mo = smp.tile([128, DHS + 1], F32, tag="mo", bufs=4)
                    nc.vector.tensor_scalar_mul(
                        mo[:], ptr2[:], rinv[:, nj2 : nj2 + 1]
                    )
                    nc.sync.dma_start(
                        send[b * N + nj2 * 128 : b * N + (nj2 + 1) * 128, :],
                        mo[:],
                    )

            if _SKIP_CC:
                dbg = d2p.tile([128, DHS + 1], F32, tag="rv")
                for r in range(RPC // 128):
                    nc.sync.dma_start(dbg[:], send[r * 128 : (r + 1) * 128, :])
                    nc.sync.dma_start(
                        y_ap[r * 128 : (r + 1) * 128, 0 : DHS + 1], dbg[:]
                    )
            else:
                # ============ Phase 2: exchange + fusion + out proj =========
                nc.gpsimd.collective_compute(
                    "AllToAll",
                    mybir.AluOpType.bypass,
                    replica_groups=[list(range(NCORES))],
                    ins=[send.opt()],
                    outs=[recv.opt()],
                )
                # recv rows: j*1024 + q*128 + p  (j = head, q = row chunk)
                recv_r = recv[:].rearrange(
                    "(j q p) d -> q p j d", j=H, q=8, p=128
                )

                rvs = []
                tsA = smp.tile([128, RPC // 128], F32, tag="tsA")
                for r in range(RPC // 128):
                    rv = d2p.tile([128, H, DHS + 1], F32, tag="rv", bufs=8,
                                  name=f"rv{r}")
                    nc.sync.dma_start(rv[:], recv_r[r])
                    rvs.append(rv)
                    tsq = smp.tile([128, H, 1], F32, tag="tsq")
                    nc.vector.tensor_mul(tsq[:], rv[:, :, 0:1], rv[:, :, 0:1])
                    nc.vector.reduce_sum(
                        tsA[:, r : r + 1], tsq[:, :, 0],
                        axis=mybir.AxisListType.X,
                    )
                # t' = exp(.5 ln(s^2 * sum_h t_h^2 + INVK*(1+H*s^2))),
                # batched over all 8 row chunks
                lnt2 = smp.tile([128, RPC // 128], F32, tag="lnt2")
                nc.scalar.activation(
                    lnt2[:], tsA[:], Ln, scale=S_CONST * S_CONST, bias=biasD[:]
                )
                tpA = smp.tile([128, RPC // 128], F32, tag="tpA")
                nc.scalar.activation(tpA[:], lnt2[:], Exp, scale=0.5)

                outts = []
                osA = smp.tile([128, RPC // 128], F32, tag="osA")
                for r in range(RPC // 128):
                    rv = rvs[r]
                    fu = d2p.tile([128, DPAD], F32, tag="fu", bufs=1)
                    nc.vector.tensor_copy(fu[:, 0:1], tpA[:, r : r + 1])
                    nc.vector.tensor_scalar_mul(
                        fu[:, 1:513].rearrange("p (j s) -> p j s", j=H),
                        rv[:, :, 1:65],
                        S_CONST,
                    )
                    nc.vector.memset(fu[:, 513:514], 1.0)
                    nc.vector.memset(fu[:, 514:DPAD], 0.0)

                    # transpose to [d, tokens] for the output contraction
                    ft = d2p.tile([128, KC, 128], BF16, tag="ft")
                    for ki in range(KC):
                        pstr = psp.tile([128, 128], F32, tag="ps")
                        nc.tensor.transpose(
                            pstr[:], fu[:, ki * 128 : (ki + 1) * 128], ident[:]
                        )
                        nc.vector.tensor_copy(ft[:, ki, :], pstr[:])

                    # output projection [128 tokens, 512]
                    pso = psp.tile([128, 512], F32, tag="ps")
                    for ki in range(KC):
                        nc.tensor.matmul(
                            pso[:],
                            ft[:, ki, :],
                            wob[:, ki, :],
                            start=(ki == 0),
                            stop=(ki == KC - 1),
                        )
                    outt = d2p.tile([128, D], F32, tag="out", bufs=8,
                                    name=f"outt{r}")
                    nc.vector.tensor_copy(outt[:, 1:D], pso[:])
                    outts.append(outt)
                    osq = smp.tile([128, 512], F32, tag="osq")
                    nc.vector.tensor_mul(osq[:], outt[:, 1:D], outt[:, 1:D])
                    nc.vector.reduce_sum(
                        osA[:, r : r + 1], osq[:], axis=mybir.AxisListType.X
                    )
                # batched t_out = exp(.5 ln(sum + INVK)), then store
                lno = smp.tile([128, RPC // 128], F32, tag="lno")
                nc.scalar.activation(lno[:], osA[:], Ln, bias=bias10[:])
                toA = smp.tile([128, RPC // 128], F32, tag="toA")
                nc.scalar.activation(toA[:], lno[:], Exp, scale=0.5)
                for r in range(RPC // 128):
                    nc.vector.tensor_copy(
                        outts[r][:, 0:1], toA[:, r : r + 1]
                    )
                    nc.sync.dma_start(
                        y_ap[r * 128 : (r + 1) * 128, :], outts[r][:]
                    )

    nc.compile()
    return nc


def _prep_inputs(x, Wq, bq, Wk, bk, Wv, bv, Wo, bo):
    xT = np.zeros((DPAD, BN), dtype=np.float32)
    xT[:D, :] = np.ascontiguousarray(x.reshape(BN, D).T)
    xT[D, :] = 1.0

    woT = np.zeros((DPAD, D - 1), dtype=np.float32)
    woT[:D + 1, :] = np.concatenate([Wo.T, bo[None, :]], axis=0)

    in_maps = []
    for h in range(NCORES):
        m = {"xT": xT, "woT": woT}
        for nm, W, bvec in (("wqT", Wq, bq), ("wkT", Wk, bk), ("wvT", Wv, bv)):
            w = np.zeros((DPAD, DHS), dtype=np.float32)
            w[0:D + 1, :] = np.concatenate([W[h].T, bvec[h][None, :]], axis=0)
            m[nm] = w
        in_maps.append(m)
    return in_maps


def _run(inputs, trace=False, **kw):
    if "nc" not in _CACHE:
        _CACHE["nc"] = _build()
    nc = _CACHE["nc"]
    in_maps = _prep_inputs(**{k: np.asarray(v) for k, v in inputs.items()})
    res = bass_utils.run_bass_kernel_spmd(
        nc, in_maps, core_ids=list(range(NCORES)), trace=trace, **kw
    )
    y = np.concatenate([res.results[c]["y"] for c in range(NCORES)], axis=0)
    return y.reshape(B, N, D), res


def kernel(**inputs):
    y, _ = _run(inputs)
    return y



# revision 14
# speedup vs baseline: 1.1088x; 1.1088x over previous
"""Lorentz multi-head attention on 8 Trainium2 NeuronCores (v2).

Sharding: head-parallel phase 1 (core c computes head c for all batches:
QKV Lorentz projections, Lorentz-inner-product scores, softmax-free
exp-attention, Lorentz-midpoint normalize). A per-batch AllToAll
(head-block -> token-block, bf16 payload) overlaps with the next batch's
phase-1 compute, and phase 2 (concat_logradius fusion + output LorentzFC)
for batch b is interleaved after phase 1 of batch b+1, so only the last
batch's exchange + fusion is exposed at the end.

Phase-2 token assignment is interleaved: core c handles tokens
[b*2048 + c*256 : b*2048 + (c+1)*256) for every batch b; the host
reassembles with a transpose.

Tricks vs v1:
- inputs (x, weights) pre-cast to bf16 on the host: halves DMA, removes
  all on-device fp32->bf16 casts of x.
- q and k projections fused into one M=128 matmul (full PE width); Wk is
  negated on the host so the Lorentz score sign flip folds into the exp
  scale (exp(-SCALE * (-score)) with k_space negated, t_k positive).
- v computed in transposed layout [65, N] like q/k, then rotated to
  token-major via 16 PE transposes (replaces 320 tiny matmuls).
- t-rows (sqrt(1/K + |s|^2)) for q, k, v batched into one Ln + one Exp on
  a [3, N] tile; rows scattered to qa/ka/vT partition 64 via SBUF DMAs.
- attention runs per query-half (1024 cols): scores psum [128, 1024],
  ONE exp per (mi, half), AV accumulates m^T in a [128, 1024] psum tile
  whose tail columns are reused for the Lorentz-radius matmuls.
- radius r = t^2 - |s|^2 computed per 128-token chunk with a single
  sign-vector matmul ([-1 x64, +1]), landing directly in token-partition
  layout for one batched Ln + Exp -> rinv.
"""

import os
import sys

sys.path.insert(0, "/opt/trn_rl_repo")

import numpy as np
import ml_dtypes

import concourse.bass as bass
import concourse.mybir as mybir
import concourse.tile as tile
from concourse import bacc, bass_utils
from concourse.masks import make_identity

# Problem constants (hardcoded per task contract)
B, N, D = 4, 2048, 513
H, DHS = 8, 64
NCORES = 8
KCURV = 0.1
INVK = 10.0
SCALE = 1.0 / np.sqrt(DHS)  # 0.125
S_CONST = 2.8479428291320801  # exp(0.5*(digamma(256)-digamma(32)))
DPAD = 640  # 513 padded to 5*128 (col 513 = constant-1 bias lane)
KC = 5  # contraction chunks of 128
BN = B * N  # 8192 tokens
RPC = BN // NCORES  # 1024 rows per core in phase 2 (256 per batch)
TPB = N // NCORES  # 256 tokens per core per batch
HALF = 1024  # query columns per attention half
F32 = mybir.dt.float32
BF16 = mybir.dt.bfloat16
Ln = mybir.ActivationFunctionType.Ln
Exp = mybir.ActivationFunctionType.Exp

_CACHE = {}
BF = ml_dtypes.bfloat16


def _patch_act_tables(nc):
    # Exp and Ln both live in the natural_log_exp_and_others set; the
    # table-load pass picks the first set containing each function, which
    # splits them across two sets and reloads tables on every Ln<->Exp
    # switch (~1.3us each). Restrict the map so the combined set wins.
    from concourse.hw_specs import get_activation_tables

    try:
        tabs = get_activation_tables(nc.m.arch)
    except Exception:
        return
    if "natural_log_exp_and_others" not in tabs:
        return
    for name, fns in tabs.items():
        if name != "natural_log_exp_and_others":
            fns.discard(Exp)
            fns.discard(Ln)


def _build():
    nc = bacc.Bacc(
        "TRN2", target_bir_lowering=False, debug=False, num_devices=NCORES
    )
    _patch_act_tables(nc)

    xT_ap = nc.dram_tensor("xT", [DPAD, BN], BF16, kind="ExternalInput").ap()
    wqkT_ap = nc.dram_tensor("wqkT", [DPAD, 128], BF16, kind="ExternalInput").ap()
    wvT_ap = nc.dram_tensor("wvT", [DPAD, DHS], BF16, kind="ExternalInput").ap()
    woT_ap = nc.dram_tensor("woT", [DPAD, D - 1], BF16, kind="ExternalInput").ap()
    y_ap = nc.dram_tensor("y", [RPC, D], F32, kind="ExternalOutput").ap()

    with tile.TileContext(nc) as tc:
        with (
            tc.tile_pool(name="const", bufs=1) as constp,
            tc.tile_pool(name="w", bufs=1) as wp,
            tc.tile_pool(name="x", bufs=1) as xp,
            tc.tile_pool(name="qk", bufs=1) as qkp,
            tc.tile_pool(name="att", bufs=1) as atp,
            tc.tile_pool(name="sm", bufs=1) as smp,
            tc.tile_pool(name="p2", bufs=1) as d2p,
            tc.tile_pool(name="ps", bufs=2, space="PSUM") as psp,
            tc.tile_pool(name="sc", bufs=2, space="PSUM") as scp,
            tc.tile_pool(name="mt", bufs=1, space="PSUM") as mtp,
            tc.tile_pool(name="dram", bufs=1, space="DRAM") as dramp,
        ):
            identB = constp.tile([128, 128], BF16)
            make_identity(nc, identB[:])
            signv = constp.tile([65, 1], BF16)
            nc.vector.memset(signv[0:64, :], -1.0)
            nc.vector.memset(signv[64:65, :], 1.0)
            selqk = constp.tile([128, 2], BF16)
            nc.vector.memset(selqk[:], 0.0)
            nc.vector.memset(selqk[0:64, 0:1], 1.0)
            nc.vector.memset(selqk[64:128, 1:2], 1.0)
            onesv = constp.tile([64, 1], BF16)
            nc.vector.memset(onesv[:], 1.0)
            bias10 = constp.tile([128, 1], F32)
            nc.vector.memset(bias10[:], INVK)
            biasD = constp.tile([128, 1], F32)
            nc.vector.memset(biasD[:], INVK * (1.0 + H * S_CONST * S_CONST))

            # Weights: [DPAD, S] viewed as [128, KC, S] (host-precast bf16)
            wqkb = wp.tile([128, KC, 128], BF16)
            wvb = wp.tile([128, KC, DHS], BF16)
            wob = wp.tile([128, KC, D - 1], BF16)
            nc.sync.dma_start(wqkb[:], wqkT_ap.rearrange("(k p) s -> p k s", p=128))
            nc.sync.dma_start(wvb[:], wvT_ap.rearrange("(k p) s -> p k s", p=128))
            nc.sync.dma_start(wob[:], woT_ap.rearrange("(k p) s -> p k s", p=128))

            sends = []
            recvs = []
            for b in range(B):
                sends.append(dramp.tile([N, DHS + 1], BF16, tag=f"send{b}",
                                        name=f"send{b}"))
                recvs.append(dramp.tile([NCORES, TPB, DHS + 1], BF16,
                                        tag=f"recv{b}", name=f"recv{b}"))

            # ---------------- Phase 1 for one batch ----------------
            def phase1(b):
                # load xT_b chunks [128, N] x 5 (bf16 direct)
                xtb = []
                for ki in range(KC):
                    t = xp.tile([128, N], BF16, tag="x", bufs=10,
                                name=f"x{b}_{ki}")
                    nc.sync.dma_start(
                        t[:],
                        xT_ap[ki * 128:(ki + 1) * 128, b * N:(b + 1) * N],
                    )
                    xtb.append(t)

                # fused qk projection + v projection + t-row sums
                qa = qkp.tile([65, N], BF16, tag="qa", bufs=2, name=f"qa{b}")
                ka = qkp.tile([65, N], BF16, tag="ka", bufs=2, name=f"ka{b}")
                vT = qkp.tile([65, N], BF16, tag="vT", bufs=2, name=f"vT{b}")
                # rows 0,1 = q,k sums; row 64 = v sums (engine SBUF writes
                # must start at partition 0/32/64; rows 2-63 are junk)
                tsta = smp.tile([65, N], F32, tag="tsta", bufs=1,
                                name=f"tsta{b}")
                for nj in range(N // 512):
                    js = slice(nj * 512, (nj + 1) * 512)
                    psqk = psp.tile([128, 512], F32, tag="ps", name=f"pqk{b}_{nj}")
                    for ki in range(KC):
                        nc.tensor.matmul(
                            psqk[:], wqkb[:, ki, :], xtb[ki][:, js],
                            start=(ki == 0), stop=(ki == KC - 1),
                        )
                    nc.vector.tensor_copy(qa[0:64, js], psqk[0:64, :])
                    nc.vector.tensor_copy(ka[0:64, js], psqk[64:128, :])
                    sqqk = smp.tile([128, 512], BF16, tag="sqqk", bufs=2,
                                    name=f"sqqk{b}_{nj}")
                    nc.vector.tensor_mul(sqqk[0:64, :], qa[0:64, js],
                                         qa[0:64, js])
                    nc.vector.tensor_mul(sqqk[64:128, :], ka[0:64, js],
                                         ka[0:64, js])
                    psv = psp.tile([64, 512], F32, tag="ps", name=f"pv{b}_{nj}")
                    for ki in range(KC):
                        nc.tensor.matmul(
                            psv[:], wvb[:, ki, :], xtb[ki][:, js],
                            start=(ki == 0), stop=(ki == KC - 1),
                        )
                    nc.vector.tensor_copy(vT[0:64, js], psv[:])
                    sqv = smp.tile([64, 512], BF16, tag="sqv", bufs=2,
                                   name=f"sqv{b}_{nj}")
                    nc.vector.tensor_mul(sqv[:], vT[0:64, js], vT[0:64, js])
                    ptr = psp.tile([65, 512], F32, tag="ps", name=f"ptr{b}_{nj}")
                    nc.tensor.matmul(ptr[0:2, :], selqk[:], sqqk[:],
                                     start=True, stop=True)
                    nc.tensor.matmul(ptr[64:65, :], onesv[:], sqv[:],
                                     start=True, stop=True)
                    nc.vector.tensor_copy(tsta[0:2, js], ptr[0:2, :])
                    nc.vector.tensor_copy(tsta[64:65, js], ptr[64:65, :])
                # t = sqrt(INVK + sum sq): one Ln + one Exp over [3, N]
                tlog = smp.tile([65, N], F32, tag="tlog", bufs=1,
                                name=f"tlog{b}")
                nc.scalar.activation(tlog[:], tsta[:], Ln,
                                     bias=bias10[0:65, :])
                trow = smp.tile([65, N], BF16, tag="trow", bufs=2,
                                name=f"trow{b}")
                nc.scalar.activation(trow[:], tlog[:], Exp, scale=0.5)
                nc.sync.dma_start(qa[64:65, :], trow[0:1, :])
                nc.sync.dma_start(ka[64:65, :], trow[1:2, :])
                nc.sync.dma_start(vT[64:65, :], trow[64:65, :])

                # rotate v to token-major [128, 16, 65]
                va = atp.tile([128, N // 128, DHS + 1], BF16, tag="va",
                              bufs=2, name=f"va{b}")
                for j in range(N // 128):
                    pstv = psp.tile([128, 65], BF16, tag="ps",
                                    name=f"pstv{b}_{j}")
                    nc.tensor.transpose(
                        pstv[:], vT[:, j * 128:(j + 1) * 128],
                        identB[0:65, 0:65],
                    )
                    nc.vector.tensor_copy(va[:, j, :], pstv[:])

                # attention per query-half
                for h2 in range(N // HALF):
                    qoff = h2 * HALF
                    mts = mtp.tile([128, HALF], F32, tag="mt", bufs=1,
                                   name=f"mts{b}_{h2}")
                    for mi in range(N // 128):
                        ks = slice(mi * 128, (mi + 1) * 128)
                        pss = scp.tile([128, HALF], F32, tag="sc", bufs=2,
                                       name=f"pss{b}_{h2}_{mi}")
                        for s in range(HALF // 512):
                            nc.tensor.matmul(
                                pss[:, s * 512:(s + 1) * 512],
                                ka[:, ks],
                                qa[:, qoff + s * 512:qoff + (s + 1) * 512],
                                start=True, stop=True,
                            )
                        pt = atp.tile([128, HALF], BF16, tag="pt", bufs=3,
                                      name=f"pt{b}_{h2}_{mi}")
                        nc.scalar.activation(pt[:], pss[:], Exp, scale=-SCALE)
                        for s in range(HALF // 512):
                            nc.tensor.matmul(
                                mts[0:65, s * 512:(s + 1) * 512],
                                va[:, mi, :],
                                pt[:, s * 512:(s + 1) * 512],
                                start=(mi == 0), stop=(mi == N // 128 - 1),
                            )

                    # drain: midpoint normalize + send
                    mTb = atp.tile([65, HALF], BF16, tag="mTb", bufs=2,
                                   name=f"mTb{b}_{h2}")
                    nc.vector.tensor_copy(mTb[:], mts[0:65, :])
                    sqb = atp.tile([65, HALF], BF16, tag="sqb", bufs=2,
                                   name=f"sqb{b}_{h2}")
                    nc.vector.tensor_mul(sqb[:], mTb[:], mTb[:])
                    # r = t^2 - |s|^2 via sign-vector matmul, token layout;
                    # reuse head of the (now consumed) mts psum tile
                    for j in range(HALF // 128):
                        nc.tensor.matmul(
                            mts[:, j:j + 1],
                            sqb[:, j * 128:(j + 1) * 128],
                            signv[:],
                            start=True, stop=True,
                        )
                    rl = smp.tile([128, HALF // 128], F32, tag="rl", bufs=2,
                                  name=f"rl{b}_{h2}")
                    nc.scalar.activation(rl[:], mts[:, 0:HALF // 128], Ln,
                                         scale=KCURV)
                    rinv = smp.tile([128, HALF // 128], F32, tag="rinv",
                                    bufs=2, name=f"rinv{b}_{h2}")
                    nc.scalar.activation(rinv[:], rl[:], Exp, scale=-0.5)
                    for g in range(HALF // 512):
                        ms = smp.tile([128, 4, DHS + 1], BF16, tag="ms",
                                      bufs=3, name=f"ms{b}_{h2}_{g}")
                        for jj in range(4):
                            j = g * 4 + jj
                            pstr = psp.tile([128, 65], BF16, tag="ps",
                                            name=f"pstr{b}_{h2}_{j}")
                            nc.tensor.transpose(
                                pstr[:], mTb[:, j * 128:(j + 1) * 128],
                                identB[0:65, 0:65],
                            )
                            nc.vector.tensor_scalar_mul(
                                ms[:, jj, :], pstr[:], rinv[:, j:j + 1]
                            )
                        dst = sends[b][qoff + g * 512:qoff + (g + 1) * 512, :]
                        nc.sync.dma_start(
                            dst.rearrange("(c p) d -> p c d", p=128), ms[:]
                        )

            # ---------------- Phase 2 for one batch ----------------
            def phase2(b):
                rvs = []
                tsA = smp.tile([128, 2], F32, tag="tsA", bufs=2,
                               name=f"tsA{b}")
                for q2 in range(2):
                    rv = d2p.tile([128, NCORES, DHS + 1], BF16, tag="rv",
                                  bufs=4, name=f"rv{b}_{q2}")
                    src = recvs[b][:, q2 * 128:(q2 + 1) * 128, :]
                    nc.sync.dma_start(rv[:], src.rearrange("j p d -> p j d"))
                    rvs.append(rv)
                    tsq = smp.tile([128, NCORES], F32, tag="tsq", bufs=2,
                                   name=f"tsq{b}_{q2}")
                    nc.vector.tensor_mul(tsq[:], rv[:, :, 64], rv[:, :, 64])
                    nc.vector.reduce_sum(tsA[:, q2:q2 + 1], tsq[:],
                                         axis=mybir.AxisListType.X)
                # t' = sqrt(s^2 * sum_h t_h^2 + INVK*(1 + H*s^2))
                lnt = smp.tile([128, 2], F32, tag="lnt", bufs=2,
                               name=f"lnt{b}")
                nc.scalar.activation(
                    lnt[:], tsA[:], Ln, scale=S_CONST * S_CONST, bias=biasD[:]
                )
                tpA = smp.tile([128, 2], F32, tag="tpA", bufs=2,
                               name=f"tpA{b}")
                nc.scalar.activation(tpA[:], lnt[:], Exp, scale=0.5)

                outts = []
                osA = smp.tile([128, 2], F32, tag="osA", bufs=2,
                               name=f"osA{b}")
                for q2 in range(2):
                    rv = rvs[q2]
                    fu = d2p.tile([128, DPAD], BF16, tag="fu", bufs=2,
                                  name=f"fu{b}_{q2}")
                    nc.vector.tensor_copy(fu[:, 0:1], tpA[:, q2:q2 + 1])
                    nc.vector.tensor_scalar_mul(
                        fu[:, 1:513].rearrange("p (j s) -> p j s", j=H),
                        rv[:, :, 0:DHS],
                        S_CONST,
                    )
                    nc.vector.memset(fu[:, 513:514], 1.0)
                    nc.vector.memset(fu[:, 514:DPAD], 0.0)

                    ftb = d2p.tile([128, KC, 128], BF16, tag="ftb", bufs=2,
                                   name=f"ftb{b}_{q2}")
                    for ki in range(KC):
                        pstf = psp.tile([128, 128], BF16, tag="ps",
                                        name=f"pstf{b}_{q2}_{ki}")
                        nc.tensor.transpose(
                            pstf[:], fu[:, ki * 128:(ki + 1) * 128], identB[:]
                        )
                        nc.vector.tensor_copy(ftb[:, ki, :], pstf[:])

                    pso = psp.tile([128, 512], F32, tag="ps",
                                   name=f"pso{b}_{q2}")
                    for ki in range(KC):
                        nc.tensor.matmul(
                            pso[:], ftb[:, ki, :], wob[:, ki, :],
                            start=(ki == 0), stop=(ki == KC - 1),
                        )
                    outt = d2p.tile([128, D], F32, tag="outt", bufs=4,
                                    name=f"outt{b}_{q2}")
                    nc.vector.tensor_copy(outt[:, 1:D], pso[:])
                    outts.append(outt)
                    osq = smp.tile([128, 512], BF16, tag="osq", bufs=2,
                                   name=f"osq{b}_{q2}")
                    nc.vector.tensor_mul(osq[:], outt[:, 1:D], outt[:, 1:D])
                    nc.vector.reduce_sum(osA[:, q2:q2 + 1], osq[:],
                                         axis=mybir.AxisListType.X)
                lno = smp.tile([128, 2], F32, tag="lno", bufs=2,
                               name=f"lno{b}")
                nc.scalar.activation(lno[:], osA[:], Ln, bias=bias10[:])
                toA = smp.tile([128, 2], F32, tag="toA", bufs=2,
                               name=f"toA{b}")
                nc.scalar.activation(toA[:], lno[:], Exp, scale=0.5)
                for q2 in range(2):
                    nc.vector.tensor_copy(outts[q2][:, 0:1], toA[:, q2:q2 + 1])
                    nc.sync.dma_start(
                        y_ap[b * TPB + q2 * 128:b * TPB + (q2 + 1) * 128, :],
                        outts[q2][:],
                    )

            # ------------- schedule: pipeline batches + A2As -------------
            for b in range(B):
                phase1(b)
                nc.gpsimd.collective_compute(
                    "AllToAll",
                    mybir.AluOpType.bypass,
                    replica_groups=[list(range(NCORES))],
                    ins=[sends[b].opt()],
                    outs=[recvs[b].opt()],
                )
                if b >= 1:
                    phase2(b - 1)
            phase2(B - 1)

    nc.compile()
    return nc


def _prep_inputs(x, Wq, bq, Wk, bk, Wv, bv, Wo, bo):
    xT = np.zeros((DPAD, BN), dtype=np.float32)
    xT[:D, :] = np.ascontiguousarray(x.reshape(BN, D).T)
    xT[D, :] = 1.0
    xTb = xT.astype(BF)

    woT = np.zeros((DPAD, D - 1), dtype=np.float32)
    woT[:D + 1, :] = np.concatenate([Wo.T, bo[None, :]], axis=0)
    woTb = woT.astype(BF)

    in_maps = []
    for h in range(NCORES):
        wqk = np.zeros((DPAD, 128), dtype=np.float32)
        wqk[:D + 1, 0:64] = np.concatenate([Wq[h].T, bq[h][None, :]], axis=0)
        # negated k: folds the Lorentz score sign into the exp scale
        wqk[:D + 1, 64:128] = -np.concatenate([Wk[h].T, bk[h][None, :]],
                                              axis=0)
        wv = np.zeros((DPAD, DHS), dtype=np.float32)
        wv[:D + 1, :] = np.concatenate([Wv[h].T, bv[h][None, :]], axis=0)
        in_maps.append({
            "xT": xTb,
            "wqkT": wqk.astype(BF),
            "wvT": wv.astype(BF),
            "woT": woTb,
        })
    return in_maps


def _run(inputs, trace=False, **kw):
    if "nc" not in _CACHE:
        _CACHE["nc"] = _build()
    nc = _CACHE["nc"]
    in_maps = _prep_inputs(**{k: np.asarray(v) for k, v in inputs.items()})
    res = bass_utils.run_bass_kernel_spmd(
        nc, in_maps, core_ids=list(range(NCORES)), trace=trace, **kw
    )
    y = np.stack([res.results[c]["y"] for c in range(NCORES)], axis=0)
    # y[c, b*256 + i, :] holds token b*2048 + c*256 + i
    y = y.reshape(NCORES, B, TPB, D).transpose(1, 0, 2, 3)
    return np.ascontiguousarray(y.reshape(B, N, D)), res


def kernel(**inputs):
    y, _ = _run(inputs)
    return y


# revision 16
# speedup vs baseline: 1.4981x; 1.3512x over previous
"""Lorentz multi-head attention on 8 Trainium2 NeuronCores (v2).

Sharding: head-parallel phase 1 (core c computes head c for all batches:
QKV Lorentz projections, Lorentz-inner-product scores, softmax-free
exp-attention, Lorentz-midpoint normalize). A per-batch AllToAll
(head-block -> token-block, bf16 payload) overlaps with the next batch's
phase-1 compute, and phase 2 (concat_logradius fusion + output LorentzFC)
for batch b is interleaved after phase 1 of batch b+1, so only the last
batch's exchange + fusion is exposed at the end.

Phase-2 token assignment is interleaved: core c handles tokens
[b*2048 + c*256 : b*2048 + (c+1)*256) for every batch b; the host
reassembles with a transpose.

Tricks vs v1:
- inputs (x, weights) pre-cast to bf16 on the host: halves DMA, removes
  all on-device fp32->bf16 casts of x.
- q and k projections fused into one M=128 matmul (full PE width); Wk is
  negated on the host so the Lorentz score sign flip folds into the exp
  scale (exp(-SCALE * (-score)) with k_space negated, t_k positive).
- v computed in transposed layout [65, N] like q/k, then rotated to
  token-major via 16 PE transposes (replaces 320 tiny matmuls).
- t-rows (sqrt(1/K + |s|^2)) for q, k, v batched into one Ln + one Exp on
  a [3, N] tile; rows scattered to qa/ka/vT partition 64 via SBUF DMAs.
- attention runs per query-half (1024 cols): scores psum [128, 1024],
  ONE exp per (mi, half), AV accumulates m^T in a [128, 1024] psum tile
  whose tail columns are reused for the Lorentz-radius matmuls.
- radius r = t^2 - |s|^2 computed per 128-token chunk with a single
  sign-vector matmul ([-1 x64, +1]), landing directly in token-partition
  layout for one batched Ln + Exp -> rinv.
"""

import os
import sys

sys.path.insert(0, "/opt/trn_rl_repo")

import numpy as np
import ml_dtypes

import concourse.bass as bass
import concourse.mybir as mybir
import concourse.tile as tile
from concourse import bacc, bass_utils
from concourse.masks import make_identity

# Problem constants (hardcoded per task contract)
B, N, D = 4, 2048, 513
H, DHS = 8, 64
NCORES = 8
KCURV = 0.1
INVK = 10.0
SCALE = 1.0 / np.sqrt(DHS)  # 0.125
S_CONST = 2.8479428291320801  # exp(0.5*(digamma(256)-digamma(32)))
DPAD = 640  # 513 padded to 5*128 (col 513 = constant-1 bias lane)
KC = 5  # contraction chunks of 128
BN = B * N  # 8192 tokens
RPC = BN // NCORES  # 1024 rows per core in phase 2 (256 per batch)
TPB = N // NCORES  # 256 tokens per core per batch
HTOK = TPB // 2  # 128 tokens per core per half-batch A2A
HALF = 1024  # query columns per attention half
F32 = mybir.dt.float32
BF16 = mybir.dt.bfloat16
Ln = mybir.ActivationFunctionType.Ln
Exp = mybir.ActivationFunctionType.Exp

_CACHE = {}
BF = ml_dtypes.bfloat16


def _patch_act_tables(nc):
    # Exp and Ln both live in the natural_log_exp_and_others set; the
    # table-load pass picks the first set containing each function, which
    # splits them across two sets and reloads tables on every Ln<->Exp
    # switch (~1.3us each). Restrict the map so the combined set wins.
    from concourse.hw_specs import get_activation_tables

    try:
        tabs = get_activation_tables(nc.m.arch)
    except Exception:
        return
    if "natural_log_exp_and_others" not in tabs:
        return
    for name, fns in tabs.items():
        if name != "natural_log_exp_and_others":
            fns.discard(Exp)
            fns.discard(Ln)


def _build():
    nc = bacc.Bacc(
        "TRN2", target_bir_lowering=False, debug=False, num_devices=NCORES
    )
    _patch_act_tables(nc)

    xT_ap = nc.dram_tensor("xT", [DPAD, BN], BF16, kind="ExternalInput").ap()
    wqkT_ap = nc.dram_tensor("wqkT", [DPAD, 128], BF16, kind="ExternalInput").ap()
    wvT_ap = nc.dram_tensor("wvT", [DPAD, DHS], BF16, kind="ExternalInput").ap()
    woT_ap = nc.dram_tensor("woT", [DPAD, D - 1], BF16, kind="ExternalInput").ap()
    y_ap = nc.dram_tensor("y", [RPC, D], F32, kind="ExternalOutput").ap()

    with tile.TileContext(nc) as tc:
        with (
            tc.tile_pool(name="const", bufs=1) as constp,
            tc.tile_pool(name="w", bufs=1) as wp,
            tc.tile_pool(name="x", bufs=1) as xp,
            tc.tile_pool(name="qk", bufs=1) as qkp,
            tc.tile_pool(name="att", bufs=1) as atp,
            tc.tile_pool(name="sm", bufs=1) as smp,
            tc.tile_pool(name="p2", bufs=1) as d2p,
            tc.tile_pool(name="ps", bufs=2, space="PSUM") as psp,
            tc.tile_pool(name="sc", bufs=2, space="PSUM") as scp,
            tc.tile_pool(name="mt", bufs=1, space="PSUM") as mtp,
            tc.tile_pool(name="dram", bufs=1, space="DRAM") as dramp,
        ):
            identB = constp.tile([128, 128], BF16)
            make_identity(nc, identB[:])
            signv = constp.tile([65, 1], BF16)
            nc.vector.memset(signv[0:64, :], -1.0)
            nc.vector.memset(signv[64:65, :], 1.0)
            # col 0 selects q rows (0-63), col 32 selects k rows (64-127):
            # activation-engine reads must start at partition 0/32/64, so
            # the k t-sum row lands on partition 32
            selqk = constp.tile([128, 33], BF16)
            nc.vector.memset(selqk[:], 0.0)
            nc.vector.memset(selqk[0:64, 0:1], 1.0)
            nc.vector.memset(selqk[64:128, 32:33], 1.0)
            onesv = constp.tile([64, 1], BF16)
            nc.vector.memset(onesv[:], 1.0)
            bias10 = constp.tile([128, 1], F32)
            nc.vector.memset(bias10[:], INVK)
            biasD = constp.tile([128, 1], F32)
            nc.vector.memset(biasD[:], INVK * (1.0 + H * S_CONST * S_CONST))

            # Weights: [DPAD, S] viewed as [128, KC, S] (host-precast bf16)
            wqkb = wp.tile([128, KC, 128], BF16)
            wvb = wp.tile([128, KC, DHS], BF16)
            wob = wp.tile([128, KC, D - 1], BF16)
            nc.sync.dma_start(wqkb[:], wqkT_ap.rearrange("(k p) s -> p k s", p=128))
            nc.sync.dma_start(wvb[:], wvT_ap.rearrange("(k p) s -> p k s", p=128))
            nc.sync.dma_start(wob[:], woT_ap.rearrange("(k p) s -> p k s", p=128))

            sends = []
            recvs = []
            for b in range(B):
                sends.append(dramp.tile([N, DHS + 1], BF16, tag=f"send{b}",
                                        name=f"send{b}"))
                recvs.append([
                    dramp.tile([NCORES, HTOK, DHS + 1], BF16,
                               tag=f"recv{b}_{h}", name=f"recv{b}_{h}")
                    for h in range(2)
                ])

            qkv = {}

            # ---- projections (q,k fused; v transposed) + t rows ----
            def proj(b):
                xtb = []
                for ki in range(KC):
                    t = xp.tile([128, N], BF16, tag="x", bufs=10,
                                name=f"x{b}_{ki}")
                    nc.sync.dma_start(
                        t[:],
                        xT_ap[ki * 128:(ki + 1) * 128, b * N:(b + 1) * N],
                    )
                    xtb.append(t)

                qa = qkp.tile([65, N], BF16, tag="qa", bufs=2, name=f"qa{b}")
                ka = qkp.tile([65, N], BF16, tag="ka", bufs=2, name=f"ka{b}")
                vT = qkp.tile([65, N], BF16, tag="vT", bufs=2, name=f"vT{b}")
                # row 0 = q sums, row 32 = k sums, row 64 = v sums
                # (partition-aligned for activation reads; rest is junk)
                tsta = smp.tile([65, N], F32, tag="tsta", bufs=2,
                                name=f"tsta{b}")
                for nj in range(N // 512):
                    js = slice(nj * 512, (nj + 1) * 512)
                    psqk = psp.tile([128, 512], F32, tag="ps", name=f"pqk{b}_{nj}")
                    for ki in range(KC):
                        nc.tensor.matmul(
                            psqk[:], wqkb[:, ki, :], xtb[ki][:, js],
                            start=(ki == 0), stop=(ki == KC - 1),
                        )
                    nc.vector.tensor_copy(qa[0:64, js], psqk[0:64, :])
                    nc.vector.tensor_copy(ka[0:64, js], psqk[64:128, :])
                    sqqk = smp.tile([128, 512], BF16, tag="sqqk", bufs=2,
                                    name=f"sqqk{b}_{nj}")
                    nc.vector.tensor_mul(sqqk[0:64, :], qa[0:64, js],
                                         qa[0:64, js])
                    nc.vector.tensor_mul(sqqk[64:128, :], ka[0:64, js],
                                         ka[0:64, js])
                    psv = psp.tile([64, 512], F32, tag="ps", name=f"pv{b}_{nj}")
                    for ki in range(KC):
                        nc.tensor.matmul(
                            psv[:], wvb[:, ki, :], xtb[ki][:, js],
                            start=(ki == 0), stop=(ki == KC - 1),
                        )
                    nc.vector.tensor_copy(vT[0:64, js], psv[:])
                    sqv = smp.tile([64, 512], BF16, tag="sqv", bufs=2,
                                   name=f"sqv{b}_{nj}")
                    nc.vector.tensor_mul(sqv[:], vT[0:64, js], vT[0:64, js])
                    ptr = psp.tile([65, 512], F32, tag="ps", name=f"ptr{b}_{nj}")
                    nc.tensor.matmul(ptr[0:33, :], selqk[:], sqqk[:],
                                     start=True, stop=True)
                    nc.tensor.matmul(ptr[64:65, :], onesv[:], sqv[:],
                                     start=True, stop=True)
                    nc.vector.tensor_copy(tsta[0:33, js], ptr[0:33, :])
                    nc.vector.tensor_copy(tsta[64:65, js], ptr[64:65, :])
                # t = sqrt(INVK + sum sq): one batched Ln, then one Exp per
                # destination row (direct writes; a DMA scatter here would
                # stall behind AllToAll traffic on the DMA engines)
                tlog = smp.tile([65, N], F32, tag="tlog", bufs=2,
                                name=f"tlog{b}")
                nc.scalar.activation(tlog[:], tsta[:], Ln,
                                     bias=bias10[0:65, :])
                nc.scalar.activation(qa[64:65, :], tlog[0:1, :], Exp,
                                     scale=0.5)
                nc.scalar.activation(ka[64:65, :], tlog[32:33, :], Exp,
                                     scale=0.5)
                nc.scalar.activation(vT[64:65, :], tlog[64:65, :], Exp,
                                     scale=0.5)

                # rotate v to token-major [128, 16, 65]
                va = atp.tile([128, N // 128, DHS + 1], BF16, tag="va",
                              bufs=2, name=f"va{b}")
                for j in range(N // 128):
                    pstv = psp.tile([128, 65], BF16, tag="ps",
                                    name=f"pstv{b}_{j}")
                    nc.tensor.transpose(
                        pstv[:], vT[:, j * 128:(j + 1) * 128],
                        identB[0:65, 0:65],
                    )
                    nc.vector.tensor_copy(va[:, j, :], pstv[:])
                qkv[b] = (qa, ka, va)

            # ---- attention + midpoint + per-half AllToAll ----
            def attention(b):
                qa, ka, va = qkv.pop(b)
                for h2 in range(N // HALF):
                    qoff = h2 * HALF
                    mts = mtp.tile([128, HALF], F32, tag="mt", bufs=1,
                                   name=f"mts{b}_{h2}")
                    # software-pipelined: scores(mi) then AV(mi-1), so the
                    # PE never sits behind an exp it is waiting on
                    prev = None
                    for mi in range(N // 128):
                        ks = slice(mi * 128, (mi + 1) * 128)
                        pss = scp.tile([128, HALF], F32, tag="sc", bufs=2,
                                       name=f"pss{b}_{h2}_{mi}")
                        for s in range(HALF // 512):
                            nc.tensor.matmul(
                                pss[:, s * 512:(s + 1) * 512],
                                ka[:, ks],
                                qa[:, qoff + s * 512:qoff + (s + 1) * 512],
                                start=True, stop=True,
                            )
                        pt = atp.tile([128, HALF], BF16, tag="pt", bufs=3,
                                      name=f"pt{b}_{h2}_{mi}")
                        nc.scalar.activation(pt[:], pss[:], Exp, scale=-SCALE)
                        if prev is not None:
                            pmi, ppt = prev
                            for s in range(HALF // 512):
                                nc.tensor.matmul(
                                    mts[0:65, s * 512:(s + 1) * 512],
                                    va[:, pmi, :],
                                    ppt[:, s * 512:(s + 1) * 512],
                                    start=(pmi == 0), stop=False,
                                )
                        prev = (mi, pt)
                    pmi, ppt = prev
                    for s in range(HALF // 512):
                        nc.tensor.matmul(
                            mts[0:65, s * 512:(s + 1) * 512],
                            va[:, pmi, :],
                            ppt[:, s * 512:(s + 1) * 512],
                            start=False, stop=True,
                        )

                    # drain: midpoint normalize + send
                    mTb = atp.tile([65, HALF], BF16, tag="mTb", bufs=2,
                                   name=f"mTb{b}_{h2}")
                    nc.vector.tensor_copy(mTb[:], mts[0:65, :])
                    sqb = atp.tile([65, HALF], BF16, tag="sqb", bufs=2,
                                   name=f"sqb{b}_{h2}")
                    nc.vector.tensor_mul(sqb[:], mTb[:], mTb[:])
                    # r = t^2 - |s|^2 via sign-vector matmul, token layout;
                    # reuse head of the (now consumed) mts psum tile
                    for j in range(HALF // 128):
                        nc.tensor.matmul(
                            mts[:, j:j + 1],
                            sqb[:, j * 128:(j + 1) * 128],
                            signv[:],
                            start=True, stop=True,
                        )
                    rl = smp.tile([128, HALF // 128], F32, tag="rl", bufs=2,
                                  name=f"rl{b}_{h2}")
                    nc.scalar.activation(rl[:], mts[:, 0:HALF // 128], Ln,
                                         scale=KCURV)
                    rinv = smp.tile([128, HALF // 128], F32, tag="rinv",
                                    bufs=2, name=f"rinv{b}_{h2}")
                    nc.scalar.activation(rinv[:], rl[:], Exp, scale=-0.5)
                    for g in range(HALF // 512):
                        ms = smp.tile([128, 4, DHS + 1], BF16, tag="ms",
                                      bufs=3, name=f"ms{b}_{h2}_{g}")
                        for jj in range(4):
                            j = g * 4 + jj
                            pstr = psp.tile([128, 65], BF16, tag="ps",
                                            name=f"pstr{b}_{h2}_{j}")
                            nc.tensor.transpose(
                                pstr[:], mTb[:, j * 128:(j + 1) * 128],
                                identB[0:65, 0:65],
                            )
                            nc.vector.tensor_scalar_mul(
                                ms[:, jj, :], pstr[:], rinv[:, j:j + 1]
                            )
                        dst = sends[b][qoff + g * 512:qoff + (g + 1) * 512, :]
                        nc.sync.dma_start(
                            dst.rearrange("(c p) d -> p c d", p=128), ms[:]
                        )
                    # exchange this half while the other half computes
                    nc.gpsimd.collective_compute(
                        "AllToAll",
                        mybir.AluOpType.bypass,
                        replica_groups=[list(range(NCORES))],
                        ins=[sends[b][qoff:qoff + HALF, :].opt()],
                        outs=[recvs[b][h2].opt()],
                    )

            # ---------------- Phase 2 for one batch ----------------
            def phase2(b):
                rvs = []
                tsA = smp.tile([128, 2], F32, tag="tsA", bufs=2,
                               name=f"tsA{b}")
                for h in range(2):
                    rv = d2p.tile([128, NCORES, DHS + 1], BF16, tag="rv",
                                  bufs=4, name=f"rv{b}_{h}")
                    nc.sync.dma_start(
                        rv[:], recvs[b][h][:].rearrange("j p d -> p j d")
                    )
                    rvs.append(rv)
                    tsq = smp.tile([128, NCORES], F32, tag="tsq", bufs=2,
                                   name=f"tsq{b}_{h}")
                    nc.vector.tensor_mul(tsq[:], rv[:, :, 64], rv[:, :, 64])
                    nc.vector.reduce_sum(tsA[:, h:h + 1], tsq[:],
                                         axis=mybir.AxisListType.X)
                # t' = sqrt(s^2 * sum_h t_h^2 + INVK*(1 + H*s^2))
                lnt = smp.tile([128, 2], F32, tag="lnt", bufs=2,
                               name=f"lnt{b}")
                nc.scalar.activation(
                    lnt[:], tsA[:], Ln, scale=S_CONST * S_CONST, bias=biasD[:]
                )
                tpA = smp.tile([128, 2], F32, tag="tpA", bufs=2,
                               name=f"tpA{b}")
                nc.scalar.activation(tpA[:], lnt[:], Exp, scale=0.5)

                outts = []
                osA = smp.tile([128, 2], F32, tag="osA", bufs=2,
                               name=f"osA{b}")
                for h in range(2):
                    rv = rvs[h]
                    fu = d2p.tile([128, DPAD], BF16, tag="fu", bufs=2,
                                  name=f"fu{b}_{h}")
                    nc.vector.tensor_copy(fu[:, 0:1], tpA[:, h:h + 1])
                    nc.vector.tensor_scalar_mul(
                        fu[:, 1:513].rearrange("p (j s) -> p j s", j=H),
                        rv[:, :, 0:DHS],
                        S_CONST,
                    )
                    nc.vector.memset(fu[:, 513:514], 1.0)
                    nc.vector.memset(fu[:, 514:DPAD], 0.0)

                    ftb = d2p.tile([128, KC, 128], BF16, tag="ftb", bufs=2,
                                   name=f"ftb{b}_{h}")
                    for ki in range(KC):
                        pstf = psp.tile([128, 128], BF16, tag="ps",
                                        name=f"pstf{b}_{h}_{ki}")
                        nc.tensor.transpose(
                            pstf[:], fu[:, ki * 128:(ki + 1) * 128], identB[:]
                        )
                        nc.vector.tensor_copy(ftb[:, ki, :], pstf[:])

                    pso = psp.tile([128, 512], F32, tag="ps",
                                   name=f"pso{b}_{h}")
                    for ki in range(KC):
                        nc.tensor.matmul(
                            pso[:], ftb[:, ki, :], wob[:, ki, :],
                            start=(ki == 0), stop=(ki == KC - 1),
                        )
                    outt = d2p.tile([128, D], F32, tag="outt", bufs=4,
                                    name=f"outt{b}_{h}")
                    nc.vector.tensor_copy(outt[:, 1:D], pso[:])
                    outts.append(outt)
                    osq = smp.tile([128, 512], BF16, tag="osq", bufs=2,
                                   name=f"osq{b}_{h}")
                    nc.vector.tensor_mul(osq[:], outt[:, 1:D], outt[:, 1:D])
                    nc.vector.reduce_sum(osA[:, h:h + 1], osq[:],
                                         axis=mybir.AxisListType.X)
                lno = smp.tile([128, 2], F32, tag="lno", bufs=2,
                               name=f"lno{b}")
                nc.scalar.activation(lno[:], osA[:], Ln, bias=bias10[:])
                toA = smp.tile([128, 2], F32, tag="toA", bufs=2,
                               name=f"toA{b}")
                nc.scalar.activation(toA[:], lno[:], Exp, scale=0.5)
                for h in range(2):
                    nc.vector.tensor_copy(outts[h][:, 0:1], toA[:, h:h + 1])
                    nc.sync.dma_start(
                        y_ap[b * TPB + h * 128:b * TPB + (h + 1) * 128, :],
                        outts[h][:],
                    )

            # ------- schedule: proj lookahead + pipelined A2A/phase2 -------
            proj(0)
            for b in range(B):
                if b + 1 < B:
                    proj(b + 1)
                attention(b)
                if b >= 1:
                    phase2(b - 1)
            phase2(B - 1)

    nc.compile()
    return nc


def _prep_inputs(x, Wq, bq, Wk, bk, Wv, bv, Wo, bo):
    xT = np.zeros((DPAD, BN), dtype=np.float32)
    xT[:D, :] = np.ascontiguousarray(x.reshape(BN, D).T)
    xT[D, :] = 1.0
    xTb = xT.astype(BF)

    woT = np.zeros((DPAD, D - 1), dtype=np.float32)
    woT[:D + 1, :] = np.concatenate([Wo.T, bo[None, :]], axis=0)
    woTb = woT.astype(BF)

    in_maps = []
    for h in range(NCORES):
        wqk = np.zeros((DPAD, 128), dtype=np.float32)
        wqk[:D + 1, 0:64] = np.concatenate([Wq[h].T, bq[h][None, :]], axis=0)
        # negated k: folds the Lorentz score sign into the exp scale
        wqk[:D + 1, 64:128] = -np.concatenate([Wk[h].T, bk[h][None, :]],
                                              axis=0)
        wv = np.zeros((DPAD, DHS), dtype=np.float32)
        wv[:D + 1, :] = np.concatenate([Wv[h].T, bv[h][None, :]], axis=0)
        in_maps.append({
            "xT": xTb,
            "wqkT": wqk.astype(BF),
            "wvT": wv.astype(BF),
            "woT": woTb,
        })
    return in_maps


def _run(inputs, trace=False, **kw):
    if "nc" not in _CACHE:
        _CACHE["nc"] = _build()
    nc = _CACHE["nc"]
    in_maps = _prep_inputs(**{k: np.asarray(v) for k, v in inputs.items()})
    res = bass_utils.run_bass_kernel_spmd(
        nc, in_maps, core_ids=list(range(NCORES)), trace=trace, **kw
    )
    y = np.stack([res.results[c]["y"] for c in range(NCORES)], axis=0)
    # y[c, b*256 + h*128 + i, :] holds token b*2048 + h*1024 + c*128 + i
    y = y.reshape(NCORES, B, 2, HTOK, D).transpose(1, 2, 0, 3, 4)
    return np.ascontiguousarray(y.reshape(B, N, D)), res


def kernel(**inputs):
    y, _ = _run(inputs)
    return y


# revision 17
# speedup vs baseline: 1.5721x; 1.0494x over previous
"""Lorentz multi-head attention on 8 Trainium2 NeuronCores (v2).

Sharding: head-parallel phase 1 (core c computes head c for all batches:
QKV Lorentz projections, Lorentz-inner-product scores, softmax-free
exp-attention, Lorentz-midpoint normalize). A per-batch AllToAll
(head-block -> token-block, bf16 payload) overlaps with the next batch's
phase-1 compute, and phase 2 (concat_logradius fusion + output LorentzFC)
for batch b is interleaved after phase 1 of batch b+1, so only the last
batch's exchange + fusion is exposed at the end.

Phase-2 token assignment is interleaved: core c handles tokens
[b*2048 + c*256 : b*2048 + (c+1)*256) for every batch b; the host
reassembles with a transpose.

Tricks vs v1:
- inputs (x, weights) pre-cast to bf16 on the host: halves DMA, removes
  all on-device fp32->bf16 casts of x.
- q and k projections fused into one M=128 matmul (full PE width); Wk is
  negated on the host so the Lorentz score sign flip folds into the exp
  scale (exp(-SCALE * (-score)) with k_space negated, t_k positive).
- v computed in transposed layout [65, N] like q/k, then rotated to
  token-major via 16 PE transposes (replaces 320 tiny matmuls).
- t-rows (sqrt(1/K + |s|^2)) for q, k, v batched into one Ln + one Exp on
  a [3, N] tile; rows scattered to qa/ka/vT partition 64 via SBUF DMAs.
- attention runs per query-half (1024 cols): scores psum [128, 1024],
  ONE exp per (mi, half), AV accumulates m^T in a [128, 1024] psum tile
  whose tail columns are reused for the Lorentz-radius matmuls.
- radius r = t^2 - |s|^2 computed per 128-token chunk with a single
  sign-vector matmul ([-1 x64, +1]), landing directly in token-partition
  layout for one batched Ln + Exp -> rinv.
"""

import os
import sys

sys.path.insert(0, "/opt/trn_rl_repo")

import numpy as np
import ml_dtypes

import concourse.bass as bass
import concourse.mybir as mybir
import concourse.tile as tile
from concourse import bacc, bass_utils
from concourse.masks import make_identity

# Problem constants (hardcoded per task contract)
B, N, D = 4, 2048, 513
H, DHS = 8, 64
NCORES = 8
KCURV = 0.1
INVK = 10.0
SCALE = 1.0 / np.sqrt(DHS)  # 0.125
S_CONST = 2.8479428291320801  # exp(0.5*(digamma(256)-digamma(32)))
DPAD = 640  # 513 padded to 5*128 (col 513 = constant-1 bias lane)
KC = 5  # contraction chunks of 128
BN = B * N  # 8192 tokens
RPC = BN // NCORES  # 1024 rows per core in phase 2 (256 per batch)
TPB = N // NCORES  # 256 tokens per core per batch
HTOK = TPB // 2  # 128 tokens per core per half-batch A2A
HALF = 1024  # query columns per attention half
F32 = mybir.dt.float32
BF16 = mybir.dt.bfloat16
Ln = mybir.ActivationFunctionType.Ln
Exp = mybir.ActivationFunctionType.Exp

_CACHE = {}
BF = ml_dtypes.bfloat16


def _patch_act_tables(nc):
    # Exp and Ln both live in the natural_log_exp_and_others set; the
    # table-load pass picks the first set containing each function, which
    # splits them across two sets and reloads tables on every Ln<->Exp
    # switch (~1.3us each). Restrict the map so the combined set wins.
    from concourse.hw_specs import get_activation_tables

    try:
        tabs = get_activation_tables(nc.m.arch)
    except Exception:
        return
    if "natural_log_exp_and_others" not in tabs:
        return
    for name, fns in tabs.items():
        if name != "natural_log_exp_and_others":
            fns.discard(Exp)
            fns.discard(Ln)


def _build():
    nc = bacc.Bacc(
        "TRN2", target_bir_lowering=False, debug=False, num_devices=NCORES
    )
    _patch_act_tables(nc)

    xT_ap = nc.dram_tensor("xT", [DPAD, BN], BF16, kind="ExternalInput").ap()
    wqkT_ap = nc.dram_tensor("wqkT", [DPAD, 128], BF16, kind="ExternalInput").ap()
    wvT_ap = nc.dram_tensor("wvT", [DPAD, DHS], BF16, kind="ExternalInput").ap()
    woT_ap = nc.dram_tensor("woT", [DPAD, D - 1], BF16, kind="ExternalInput").ap()
    y_ap = nc.dram_tensor("y", [RPC, D], F32, kind="ExternalOutput").ap()

    with tile.TileContext(nc) as tc:
        with (
            tc.tile_pool(name="const", bufs=1) as constp,
            tc.tile_pool(name="w", bufs=1) as wp,
            tc.tile_pool(name="x", bufs=1) as xp,
            tc.tile_pool(name="qk", bufs=1) as qkp,
            tc.tile_pool(name="att", bufs=1) as atp,
            tc.tile_pool(name="sm", bufs=1) as smp,
            tc.tile_pool(name="p2", bufs=1) as d2p,
            tc.tile_pool(name="ps", bufs=2, space="PSUM") as psp,
            tc.tile_pool(name="sc", bufs=2, space="PSUM") as scp,
            tc.tile_pool(name="mt", bufs=1, space="PSUM") as mtp,
            tc.tile_pool(name="dram", bufs=1, space="DRAM") as dramp,
        ):
            identB = constp.tile([128, 128], BF16)
            make_identity(nc, identB[:])
            signv = constp.tile([65, 1], BF16)
            nc.vector.memset(signv[0:64, :], -1.0)
            nc.vector.memset(signv[64:65, :], 1.0)
            # col 0 selects q rows (0-63), col 32 selects k rows (64-127):
            # activation-engine reads must start at partition 0/32/64, so
            # the k t-sum row lands on partition 32
            selqk = constp.tile([128, 33], BF16)
            nc.vector.memset(selqk[:], 0.0)
            nc.vector.memset(selqk[0:64, 0:1], 1.0)
            nc.vector.memset(selqk[64:128, 32:33], 1.0)
            onesv = constp.tile([64, 1], BF16)
            nc.vector.memset(onesv[:], 1.0)
            bias10 = constp.tile([128, 1], F32)
            nc.vector.memset(bias10[:], INVK)
            biasD = constp.tile([128, 1], F32)
            nc.vector.memset(biasD[:], INVK * (1.0 + H * S_CONST * S_CONST))

            # Weights: [DPAD, S] viewed as [128, KC, S] (host-precast bf16)
            wqkb = wp.tile([128, KC, 128], BF16)
            wvb = wp.tile([128, KC, DHS], BF16)
            wob = wp.tile([128, KC, D - 1], BF16)
            nc.sync.dma_start(wqkb[:], wqkT_ap.rearrange("(k p) s -> p k s", p=128))
            nc.sync.dma_start(wvb[:], wvT_ap.rearrange("(k p) s -> p k s", p=128))
            nc.sync.dma_start(wob[:], woT_ap.rearrange("(k p) s -> p k s", p=128))

            sends = []
            recvs = []
            for b in range(B):
                sends.append(dramp.tile([N, DHS + 1], BF16, tag=f"send{b}",
                                        name=f"send{b}"))
                recvs.append([
                    dramp.tile([NCORES, HTOK, DHS + 1], BF16,
                               tag=f"recv{b}_{h}", name=f"recv{b}_{h}")
                    for h in range(2)
                ])

            qkv = {}
            xts = {}

            # x loads are issued well ahead of each batch so they never
            # queue behind AllToAll traffic on the DMA engines
            def xload(b):
                xtb = []
                for ki in range(KC):
                    t = xp.tile([128, N], BF16, tag="x", bufs=15,
                                name=f"x{b}_{ki}")
                    nc.sync.dma_start(
                        t[:],
                        xT_ap[ki * 128:(ki + 1) * 128, b * N:(b + 1) * N],
                    )
                    xtb.append(t)
                xts[b] = xtb

            # ---- projections (q,k fused; v transposed) + t rows ----
            def proj(b):
                xtb = xts.pop(b)

                qa = qkp.tile([65, N], BF16, tag="qa", bufs=2, name=f"qa{b}")
                ka = qkp.tile([65, N], BF16, tag="ka", bufs=2, name=f"ka{b}")
                vT = qkp.tile([65, N], BF16, tag="vT", bufs=2, name=f"vT{b}")
                # row 0 = q sums, row 32 = k sums, row 64 = v sums
                # (partition-aligned for activation reads; rest is junk)
                tsta = smp.tile([65, N], F32, tag="tsta", bufs=2,
                                name=f"tsta{b}")
                for nj in range(N // 512):
                    js = slice(nj * 512, (nj + 1) * 512)
                    psqk = psp.tile([128, 512], F32, tag="ps", name=f"pqk{b}_{nj}")
                    for ki in range(KC):
                        nc.tensor.matmul(
                            psqk[:], wqkb[:, ki, :], xtb[ki][:, js],
                            start=(ki == 0), stop=(ki == KC - 1),
                        )
                    nc.vector.tensor_copy(qa[0:64, js], psqk[0:64, :])
                    nc.vector.tensor_copy(ka[0:64, js], psqk[64:128, :])
                    sqqk = smp.tile([128, 512], BF16, tag="sqqk", bufs=2,
                                    name=f"sqqk{b}_{nj}")
                    nc.vector.tensor_mul(sqqk[0:64, :], qa[0:64, js],
                                         qa[0:64, js])
                    nc.vector.tensor_mul(sqqk[64:128, :], ka[0:64, js],
                                         ka[0:64, js])
                    psv = psp.tile([64, 512], F32, tag="ps", name=f"pv{b}_{nj}")
                    for ki in range(KC):
                        nc.tensor.matmul(
                            psv[:], wvb[:, ki, :], xtb[ki][:, js],
                            start=(ki == 0), stop=(ki == KC - 1),
                        )
                    nc.vector.tensor_copy(vT[0:64, js], psv[:])
                    sqv = smp.tile([64, 512], BF16, tag="sqv", bufs=2,
                                   name=f"sqv{b}_{nj}")
                    nc.vector.tensor_mul(sqv[:], vT[0:64, js], vT[0:64, js])
                    ptr = psp.tile([65, 512], F32, tag="ps", name=f"ptr{b}_{nj}")
                    nc.tensor.matmul(ptr[0:33, :], selqk[:], sqqk[:],
                                     start=True, stop=True)
                    nc.tensor.matmul(ptr[64:65, :], onesv[:], sqv[:],
                                     start=True, stop=True)
                    nc.vector.tensor_copy(tsta[0:33, js], ptr[0:33, :])
                    nc.vector.tensor_copy(tsta[64:65, js], ptr[64:65, :])
                # t = sqrt(INVK + sum sq): one batched Ln, then one Exp per
                # destination row (direct writes; a DMA scatter here would
                # stall behind AllToAll traffic on the DMA engines)
                tlog = smp.tile([65, N], F32, tag="tlog", bufs=2,
                                name=f"tlog{b}")
                nc.scalar.activation(tlog[:], tsta[:], Ln,
                                     bias=bias10[0:65, :])
                nc.scalar.activation(qa[64:65, :], tlog[0:1, :], Exp,
                                     scale=0.5)
                nc.scalar.activation(ka[64:65, :], tlog[32:33, :], Exp,
                                     scale=0.5)
                nc.scalar.activation(vT[64:65, :], tlog[64:65, :], Exp,
                                     scale=0.5)

                # rotate v to token-major [128, 16, 65]
                va = atp.tile([128, N // 128, DHS + 1], BF16, tag="va",
                              bufs=2, name=f"va{b}")
                for j in range(N // 128):
                    pstv = psp.tile([128, 65], BF16, tag="ps",
                                    name=f"pstv{b}_{j}")
                    nc.tensor.transpose(
                        pstv[:], vT[:, j * 128:(j + 1) * 128],
                        identB[0:65, 0:65],
                    )
                    nc.vector.tensor_copy(va[:, j, :], pstv[:])
                qkv[b] = (qa, ka, va)

            # ---- attention + midpoint + per-half AllToAll ----
            def attention(b):
                qa, ka, va = qkv.pop(b)
                for h2 in range(N // HALF):
                    qoff = h2 * HALF
                    mts = mtp.tile([128, HALF], F32, tag="mt", bufs=1,
                                   name=f"mts{b}_{h2}")
                    # software-pipelined: scores(mi) then AV(mi-1), so the
                    # PE never sits behind an exp it is waiting on
                    prev = None
                    for mi in range(N // 128):
                        ks = slice(mi * 128, (mi + 1) * 128)
                        pss = scp.tile([128, HALF], F32, tag="sc", bufs=2,
                                       name=f"pss{b}_{h2}_{mi}")
                        for s in range(HALF // 512):
                            nc.tensor.matmul(
                                pss[:, s * 512:(s + 1) * 512],
                                ka[:, ks],
                                qa[:, qoff + s * 512:qoff + (s + 1) * 512],
                                start=True, stop=True,
                            )
                        pt = atp.tile([128, HALF], BF16, tag="pt", bufs=3,
                                      name=f"pt{b}_{h2}_{mi}")
                        nc.scalar.activation(pt[:], pss[:], Exp, scale=-SCALE)
                        if prev is not None:
                            pmi, ppt = prev
                            for s in range(HALF // 512):
                                nc.tensor.matmul(
                                    mts[0:65, s * 512:(s + 1) * 512],
                                    va[:, pmi, :],
                                    ppt[:, s * 512:(s + 1) * 512],
                                    start=(pmi == 0), stop=False,
                                )
                        prev = (mi, pt)
                    pmi, ppt = prev
                    for s in range(HALF // 512):
                        nc.tensor.matmul(
                            mts[0:65, s * 512:(s + 1) * 512],
                            va[:, pmi, :],
                            ppt[:, s * 512:(s + 1) * 512],
                            start=False, stop=True,
                        )

                    # drain: midpoint normalize + send
                    mTb = atp.tile([65, HALF], BF16, tag="mTb", bufs=2,
                                   name=f"mTb{b}_{h2}")
                    nc.vector.tensor_copy(mTb[:], mts[0:65, :])
                    sqb = atp.tile([65, HALF], BF16, tag="sqb", bufs=2,
                                   name=f"sqb{b}_{h2}")
                    nc.vector.tensor_mul(sqb[:], mTb[:], mTb[:])
                    # r = t^2 - |s|^2 via sign-vector matmul, token layout;
                    # reuse head of the (now consumed) mts psum tile
                    for j in range(HALF // 128):
                        nc.tensor.matmul(
                            mts[:, j:j + 1],
                            sqb[:, j * 128:(j + 1) * 128],
                            signv[:],
                            start=True, stop=True,
                        )
                    rl = smp.tile([128, HALF // 128], F32, tag="rl", bufs=2,
                                  name=f"rl{b}_{h2}")
                    nc.scalar.activation(rl[:], mts[:, 0:HALF // 128], Ln,
                                         scale=KCURV)
                    rinv = smp.tile([128, HALF // 128], F32, tag="rinv",
                                    bufs=2, name=f"rinv{b}_{h2}")
                    nc.scalar.activation(rinv[:], rl[:], Exp, scale=-0.5)
                    for g in range(HALF // 512):
                        ms = smp.tile([128, 4, DHS + 1], BF16, tag="ms",
                                      bufs=3, name=f"ms{b}_{h2}_{g}")
                        for jj in range(4):
                            j = g * 4 + jj
                            pstr = psp.tile([128, 65], BF16, tag="ps",
                                            name=f"pstr{b}_{h2}_{j}")
                            nc.tensor.transpose(
                                pstr[:], mTb[:, j * 128:(j + 1) * 128],
                                identB[0:65, 0:65],
                            )
                            nc.vector.tensor_scalar_mul(
                                ms[:, jj, :], pstr[:], rinv[:, j:j + 1]
                            )
                        dst = sends[b][qoff + g * 512:qoff + (g + 1) * 512, :]
                        nc.sync.dma_start(
                            dst.rearrange("(c p) d -> p c d", p=128), ms[:]
                        )
                    # exchange this half while the other half computes
                    nc.gpsimd.collective_compute(
                        "AllToAll",
                        mybir.AluOpType.bypass,
                        replica_groups=[list(range(NCORES))],
                        ins=[sends[b][qoff:qoff + HALF, :].opt()],
                        outs=[recvs[b][h2].opt()],
                    )

            # ---------------- Phase 2 for one batch ----------------
            def phase2(b):
                rvs = []
                tsA = smp.tile([128, 2], F32, tag="tsA", bufs=2,
                               name=f"tsA{b}")
                for h in range(2):
                    rv = d2p.tile([128, NCORES, DHS + 1], BF16, tag="rv",
                                  bufs=4, name=f"rv{b}_{h}")
                    nc.sync.dma_start(
                        rv[:], recvs[b][h][:].rearrange("j p d -> p j d")
                    )
                    rvs.append(rv)
                    tsq = smp.tile([128, NCORES], F32, tag="tsq", bufs=2,
                                   name=f"tsq{b}_{h}")
                    nc.vector.tensor_mul(tsq[:], rv[:, :, 64], rv[:, :, 64])
                    nc.vector.reduce_sum(tsA[:, h:h + 1], tsq[:],
                                         axis=mybir.AxisListType.X)
                # t' = sqrt(s^2 * sum_h t_h^2 + INVK*(1 + H*s^2))
                lnt = smp.tile([128, 2], F32, tag="lnt", bufs=2,
                               name=f"lnt{b}")
                nc.scalar.activation(
                    lnt[:], tsA[:], Ln, scale=S_CONST * S_CONST, bias=biasD[:]
                )
                tpA = smp.tile([128, 2], F32, tag="tpA", bufs=2,
                               name=f"tpA{b}")
                nc.scalar.activation(tpA[:], lnt[:], Exp, scale=0.5)

                outts = []
                osA = smp.tile([128, 2], F32, tag="osA", bufs=2,
                               name=f"osA{b}")
                for h in range(2):
                    rv = rvs[h]
                    fu = d2p.tile([128, DPAD], BF16, tag="fu", bufs=2,
                                  name=f"fu{b}_{h}")
                    nc.vector.tensor_copy(fu[:, 0:1], tpA[:, h:h + 1])
                    nc.vector.tensor_scalar_mul(
                        fu[:, 1:513].rearrange("p (j s) -> p j s", j=H),
                        rv[:, :, 0:DHS],
                        S_CONST,
                    )
                    nc.vector.memset(fu[:, 513:514], 1.0)
                    nc.vector.memset(fu[:, 514:DPAD], 0.0)

                    ftb = d2p.tile([128, KC, 128], BF16, tag="ftb", bufs=2,
                                   name=f"ftb{b}_{h}")
                    for ki in range(KC):
                        pstf = psp.tile([128, 128], BF16, tag="ps",
                                        name=f"pstf{b}_{h}_{ki}")
                        nc.tensor.transpose(
                            pstf[:], fu[:, ki * 128:(ki + 1) * 128], identB[:]
                        )
                        nc.vector.tensor_copy(ftb[:, ki, :], pstf[:])

                    pso = psp.tile([128, 512], F32, tag="ps",
                                   name=f"pso{b}_{h}")
                    for ki in range(KC):
                        nc.tensor.matmul(
                            pso[:], ftb[:, ki, :], wob[:, ki, :],
                            start=(ki == 0), stop=(ki == KC - 1),
                        )
                    outt = d2p.tile([128, D], F32, tag="outt", bufs=4,
                                    name=f"outt{b}_{h}")
                    nc.vector.tensor_copy(outt[:, 1:D], pso[:])
                    outts.append(outt)
                    osq = smp.tile([128, 512], BF16, tag="osq", bufs=2,
                                   name=f"osq{b}_{h}")
                    nc.vector.tensor_mul(osq[:], outt[:, 1:D], outt[:, 1:D])
                    nc.vector.reduce_sum(osA[:, h:h + 1], osq[:],
                                         axis=mybir.AxisListType.X)
                lno = smp.tile([128, 2], F32, tag="lno", bufs=2,
                               name=f"lno{b}")
                nc.scalar.activation(lno[:], osA[:], Ln, bias=bias10[:])
                toA = smp.tile([128, 2], F32, tag="toA", bufs=2,
                               name=f"toA{b}")
                nc.scalar.activation(toA[:], lno[:], Exp, scale=0.5)
                for h in range(2):
                    nc.vector.tensor_copy(outts[h][:, 0:1], toA[:, h:h + 1])
                    nc.sync.dma_start(
                        y_ap[b * TPB + h * 128:b * TPB + (h + 1) * 128, :],
                        outts[h][:],
                    )

            # ------- schedule: proj lookahead + pipelined A2A/phase2 -------
            xload(0)
            xload(1)
            proj(0)
            for b in range(B):
                if b + 2 < B:
                    xload(b + 2)
                if b + 1 < B:
                    proj(b + 1)
                attention(b)
                if b >= 1:
                    phase2(b - 1)
            phase2(B - 1)

    nc.compile()
    return nc


def _prep_inputs(x, Wq, bq, Wk, bk, Wv, bv, Wo, bo):
    xT = np.zeros((DPAD, BN), dtype=np.float32)
    xT[:D, :] = np.ascontiguousarray(x.reshape(BN, D).T)
    xT[D, :] = 1.0
    xTb = xT.astype(BF)

    woT = np.zeros((DPAD, D - 1), dtype=np.float32)
    woT[:D + 1, :] = np.concatenate([Wo.T, bo[None, :]], axis=0)
    woTb = woT.astype(BF)

    in_maps = []
    for h in range(NCORES):
        wqk = np.zeros((DPAD, 128), dtype=np.float32)
        wqk[:D + 1, 0:64] = np.concatenate([Wq[h].T, bq[h][None, :]], axis=0)
        # negated k: folds the Lorentz score sign into the exp scale
        wqk[:D + 1, 64:128] = -np.concatenate([Wk[h].T, bk[h][None, :]],
                                              axis=0)
        wv = np.zeros((DPAD, DHS), dtype=np.float32)
        wv[:D + 1, :] = np.concatenate([Wv[h].T, bv[h][None, :]], axis=0)
        in_maps.append({
            "xT": xTb,
            "wqkT": wqk.astype(BF),
            "wvT": wv.astype(BF),
            "woT": woTb,
        })
    return in_maps


def _run(inputs, trace=False, **kw):
    if "nc" not in _CACHE:
        _CACHE["nc"] = _build()
    nc = _CACHE["nc"]
    in_maps = _prep_inputs(**{k: np.asarray(v) for k, v in inputs.items()})
    res = bass_utils.run_bass_kernel_spmd(
        nc, in_maps, core_ids=list(range(NCORES)), trace=trace, **kw
    )
    y = np.stack([res.results[c]["y"] for c in range(NCORES)], axis=0)
    # y[c, b*256 + h*128 + i, :] holds token b*2048 + h*1024 + c*128 + i
    y = y.reshape(NCORES, B, 2, HTOK, D).transpose(1, 2, 0, 3, 4)
    return np.ascontiguousarray(y.reshape(B, N, D)), res


def kernel(**inputs):
    y, _ = _run(inputs)
    return y
